# revision 1
# baseline (speedup 1.0000x reference)
"""Trainium2 Bass kernel for a dense transformer block (B=4, S=2048, D=2048,
H=16, HD=128, FFN 4x) on 8 NeuronCores.

Sharding: head-parallel attention (2 heads/core), one AllToAll
(scatter tokens / gather heads), then token-parallel wo + FFN
(1024 tokens/core). Matmuls in bf16 (fp32 matmul is 4 cyc/row on the PE),
fp32 PSUM accumulation, softmax/residual math in fp32.
RMSNorm gains g1/g2 are folded into wq/wk/wv/w1 on the host.
"""

import numpy as np
import ml_dtypes

import concourse.bass as bass
import concourse.tile as tile
import concourse.mybir as mybir
from concourse import bacc
from concourse.bass_utils import run_bass_kernel_spmd

F32 = mybir.dt.float32
BF16 = mybir.dt.bfloat16
AF = mybir.ActivationFunctionType

NC_CORES = 8


class Cfg:
    def __init__(self, B=4, S=2048, D=2048, H=16, HD=128, DFF=8192):
        self.B, self.S, self.D, self.H, self.HD, self.DFF = B, S, D, H, HD, DFF
        self.T = B * S                      # total tokens
        self.MYT = self.T // NC_CORES       # tokens per core
        self.HPC = H // NC_CORES            # heads per core
        self.KD = D // 128                  # D partition-tiles
        self.FT = DFF // 128                # ffn partition-tiles
        assert self.HPC * NC_CORES == H and HD == 128 and D == H * HD
        assert self.MYT % 128 == 0 and S % 512 == 0 and DFF % 512 == 0
        self.TOKB = min(512, self.MYT)      # ffn token-block
        self.NBLK = self.MYT // self.TOKB


def build_nc(cfg: Cfg):
    B, S, D, HD, DFF = cfg.B, cfg.S, cfg.D, cfg.HD, cfg.DFF
    T, MYT, HPC, KD, FT = cfg.T, cfg.MYT, cfg.HPC, cfg.KD, cfg.FT
    TOKB, NBLK = cfg.TOKB, cfg.NBLK
    H = cfg.H
    SCALE = float(HD) ** -0.5
    import contextlib

    nc = bacc.Bacc("TRN2", target_bir_lowering=False, debug=False,
                   num_devices=NC_CORES)

    x_ext = nc.dram_tensor("x", [T, D], F32, kind="ExternalInput")
    xm_ext = nc.dram_tensor("xmine", [MYT, D], F32, kind="ExternalInput")
    wq_ext = nc.dram_tensor("wq", [128, HPC, KD, HD], BF16, kind="ExternalInput")
    wk_ext = nc.dram_tensor("wk", [128, HPC, KD, HD], BF16, kind="ExternalInput")
    wv_ext = nc.dram_tensor("wv", [128, HPC, KD, HD], BF16, kind="ExternalInput")
    wo_ext = nc.dram_tensor("wo", [128, KD, D], BF16, kind="ExternalInput")
    w1_ext = nc.dram_tensor("w1", [D, DFF], BF16, kind="ExternalInput")
    w2_ext = nc.dram_tensor("w2", [DFF, D], BF16, kind="ExternalInput")
    b1_ext = nc.dram_tensor("b1c", [128, FT], F32, kind="ExternalInput")
    b2_ext = nc.dram_tensor("b2bc", [128, D], F32, kind="ExternalInput")
    mask_ext = nc.dram_tensor("maskbig", [128, 896], BF16, kind="ExternalInput")
    out_ext = nc.dram_tensor("out", [MYT, D], F32, kind="ExternalOutput")

    MYB = MYT // B          # my tokens per batch
    a2a_in = [nc.dram_tensor(f"a2a_in{b}", [NC_CORES, HPC, 128, MYB], BF16)
              for b in range(B)]
    a2a_out = [nc.dram_tensor(f"a2a_out{b}", [NC_CORES, HPC, 128, MYB], BF16)
               for b in range(B)]
    h1_dram = nc.dram_tensor("h1_spill", [T, D], BF16)
    h2_dram = nc.dram_tensor("h2_spill", [MYT, D], BF16)
    x1_dram = nc.dram_tensor("x1_spill", [MYT, D], F32)

    with tile.TileContext(nc) as tc:
        ctx = contextlib.ExitStack()
        const = ctx.enter_context(tc.tile_pool(name="const", bufs=1))

        mask_sb = const.tile([128, 896], BF16)
        nc.sync.dma_start(out=mask_sb, in_=mask_ext[:])
        ones_sb = const.tile([128, 128], BF16)
        nc.vector.memset(ones_sb, 1.0)
        eps_sb = const.tile([128, 1], F32)
        nc.vector.memset(eps_sb, 1e-8)
        b1_sb = const.tile([128, FT], F32)
        nc.sync.dma_start(out=b1_sb, in_=b1_ext[:])

        # h2T blocks survive the wo phase into the FFN phase
        h2T_pool = ctx.enter_context(tc.tile_pool(name="h2T", bufs=NBLK))

        # ---------------- Phase A: norm1 + QKV + attention ----------------
        pa = contextlib.ExitStack()
        qkvw_pool = pa.enter_context(tc.tile_pool(name="qkvw", bufs=1))
        aout_pool = pa.enter_context(tc.tile_pool(name="aout", bufs=2 * HPC))
        xa_pool = pa.enter_context(tc.tile_pool(name="xa", bufs=2))
        scr_pool = pa.enter_context(tc.tile_pool(name="scr", bufs=2))
        stat_pool = pa.enter_context(tc.tile_pool(name="stat", bufs=3))
        h1_pool = pa.enter_context(tc.tile_pool(name="h1", bufs=2))
        h1T_pool = pa.enter_context(tc.tile_pool(name="h1T", bufs=3))
        qkv_pool = pa.enter_context(tc.tile_pool(name="qkvT", bufs=2))
        v_pool = pa.enter_context(tc.tile_pool(name="vtok", bufs=4))
        p_pool = pa.enter_context(tc.tile_pool(name="pT", bufs=3))
        pm_pool = pa.enter_context(tc.tile_pool(name="pTm", bufs=3))
        dd_pool = pa.enter_context(tc.tile_pool(name="dd", bufs=2))
        ps_qkv = pa.enter_context(tc.tile_pool(name="psqkv", bufs=2, space="PSUM"))
        ps_v = pa.enter_context(tc.tile_pool(name="psv", bufs=1, space="PSUM"))
        ps_sc = pa.enter_context(tc.tile_pool(name="pssc", bufs=2, space="PSUM"))
        ps_av = pa.enter_context(tc.tile_pool(name="psav", bufs=2, space="PSUM"))
        ps_den = pa.enter_context(tc.tile_pool(name="psden", bufs=1, space="PSUM"))
        if True:
            wq_sb = qkvw_pool.tile([128, HPC, KD, HD], BF16)
            wk_sb = qkvw_pool.tile([128, HPC, KD, HD], BF16)
            wv_sb = qkvw_pool.tile([128, HPC, KD, HD], BF16)
            nc.sync.dma_start(out=wq_sb, in_=wq_ext[:])
            nc.sync.dma_start(out=wk_sb, in_=wk_ext[:])
            nc.sync.dma_start(out=wv_sb, in_=wv_ext[:])
            for b in range(B):
                attn_outT = [aout_pool.tile([128, S], BF16, tag="attn_outT",
                                            name="attn_outT")
                             for _ in range(HPC)]
                qT = [qkv_pool.tile([128, S], BF16, tag="qT", name="qT")
                      for _ in range(HPC)]
                kT = [qkv_pool.tile([128, S], BF16, tag="kT", name="kT")
                      for _ in range(HPC)]
                v_sb = [v_pool.tile([128, S // 128, 128], BF16, tag="v",
                                    name="v") for _ in range(HPC)]
                for nb in range(S // 512):
                    tok0 = b * S + nb * 512
                    for t4 in range(4):
                        row0 = tok0 + t4 * 128
                        x_sb = xa_pool.tile([128, D], F32)
                        nc.sync.dma_start(out=x_sb, in_=x_ext[row0:row0 + 128, :])
                        scr = scr_pool.tile([128, D], BF16)
                        ssq = stat_pool.tile([128, 1], F32, tag="ssq")
                        nc.scalar.activation(scr, x_sb, AF.Square, accum_out=ssq)
                        ms = stat_pool.tile([128, 1], F32, tag="ms")
                        nc.scalar.activation(ms, ssq, AF.Identity,
                                             bias=eps_sb[:, 0:1], scale=1.0 / D)
                        inv = stat_pool.tile([128, 1], F32, tag="inv")
                        nc.vector.reciprocal(inv, ms)
                        r = stat_pool.tile([128, 1], F32, tag="r")
                        nc.scalar.activation(r, inv, AF.Sqrt)
                        h1 = h1_pool.tile([128, D], BF16)
                        nc.scalar.activation(h1, x_sb, AF.Copy, scale=r[:, 0:1])
                        nc.sync.dma_start(out=h1_dram[row0:row0 + 128, :], in_=h1)
                    # big DRAM-side transposes: [512, 128] -> [128, 512]
                    h1T = h1T_pool.tile([128, KD, 512], BF16)
                    for d in range(KD):
                        nc.scalar.dma_start(
                            out=h1T[:, d, :],
                            in_=h1_dram[tok0:tok0 + 512,
                                        d * 128:(d + 1) * 128], transpose=True)
                    for h in range(HPC):
                        for wsb, dstT in ((wq_sb, qT), (wk_sb, kT)):
                            ps = ps_qkv.tile([128, 512], F32)
                            for kd in range(KD):
                                nc.tensor.matmul(ps, wsb[:, h, kd, :],
                                                 h1T[:, kd, :],
                                                 start=(kd == 0),
                                                 stop=(kd == KD - 1))
                            nc.scalar.copy(
                                dstT[h][:, nb * 512:(nb + 1) * 512], ps)
                    # v token-major via matmul: both heads as one moving block
                    for sk4 in range(4):
                        psv = ps_v.tile([128, HPC * HD], F32)
                        for kd in range(KD):
                            nc.tensor.matmul(
                                psv, h1T[:, kd, sk4 * 128:(sk4 + 1) * 128],
                                wv_sb[:, :, kd, :],
                                start=(kd == 0), stop=(kd == KD - 1))
                        for h in range(HPC):
                            nc.vector.tensor_copy(
                                v_sb[h][:, nb * 4 + sk4, :],
                                psv[:, h * HD:(h + 1) * HD])
                for h in range(HPC):
                    for q4 in range(S // 512):
                        sq_off = q4 * 512
                        av_ps = ps_av.tile([128, 512], F32)
                        den_ps = ps_den.tile([128, 512], F32)
                        n_alive = q4 * 4 + 4
                        for sk in range(n_alive):
                            sc_ps = ps_sc.tile([128, 512], F32)
                            nc.tensor.matmul(
                                sc_ps, kT[h][:, sk * 128:(sk + 1) * 128],
                                qT[h][:, sq_off:sq_off + 512],
                                start=True, stop=True)
                            pT = p_pool.tile([128, 512], BF16)
                            nc.scalar.activation(pT, sc_ps, AF.Exp, scale=SCALE)
                            use = pT
                            delta = sq_off - sk * 128
                            if delta < 128:  # partial (diag-band) tile
                                o = delta + 384
                                pTm = pm_pool.tile([128, 512], BF16)
                                nc.vector.tensor_mul(pTm, pT,
                                                     mask_sb[:, o:o + 512])
                                use = pTm
                            nc.tensor.matmul(av_ps, v_sb[h][:, sk, :], use,
                                             start=(sk == 0),
                                             stop=(sk == n_alive - 1))
                            nc.tensor.matmul(den_ps, ones_sb, use,
                                             start=(sk == 0),
                                             stop=(sk == n_alive - 1))
                        dinv = dd_pool.tile([128, 512], F32, tag="dinv")
                        nc.vector.reciprocal(dinv, den_ps)
                        nc.vector.tensor_mul(
                            attn_outT[h][:, sq_off:sq_off + 512],
                            av_ps, dinv)

                # per-batch exchange: overlaps with next batch's attention
                for h in range(HPC):
                    for j in range(NC_CORES):
                        nc.sync.dma_start(
                            out=a2a_in[b][j, h],
                            in_=attn_outT[h][:, j * MYB:(j + 1) * MYB])
                nc.gpsimd.collective_compute(
                    "AllToAll", mybir.AluOpType.bypass,
                    replica_groups=[list(range(NC_CORES))],
                    ins=[a2a_in[b][:].opt()], outs=[a2a_out[b][:].opt()])
        pa.close()

        # ---------------- Phase B: wo + residual + norm2 + FFN -------------
        a2a_flat = [a2a_out[b].ap().rearrange("c h p t -> (c h) p t")
                    for b in range(B)]
        pb = contextlib.ExitStack()
        acat_pool = pb.enter_context(tc.tile_pool(name="acat", bufs=1))
        wo_pool = pb.enter_context(tc.tile_pool(name="wo", bufs=1))
        xb_pool = pb.enter_context(tc.tile_pool(name="xb", bufs=2))
        x1_pool = pb.enter_context(tc.tile_pool(name="x1", bufs=2))
        stat2_pool = pb.enter_context(tc.tile_pool(name="stat2", bufs=3))
        scr2_pool = pb.enter_context(tc.tile_pool(name="scr2", bufs=2))
        h2_pool = pb.enter_context(tc.tile_pool(name="h2", bufs=2))
        ps_wo = pb.enter_context(tc.tile_pool(name="pswo", bufs=2, space="PSUM"))
        if True:
            attn_catT = acat_pool.tile([128, H, MYT], BF16)
            for b in range(B):
                for kc in range(H):
                    nc.sync.dma_start(
                        out=attn_catT[:, kc, b * MYB:(b + 1) * MYB],
                        in_=a2a_flat[b][kc])
            wo_sb = wo_pool.tile([128, KD, D], BF16)
            nc.sync.dma_start(out=wo_sb, in_=wo_ext[:])

            for m in range(MYT // 128):
                xm_sb = xb_pool.tile([128, D], F32)
                nc.sync.dma_start(out=xm_sb, in_=xm_ext[m * 128:(m + 1) * 128, :])
                x1_sb = x1_pool.tile([128, D], F32)
                for n in range(D // 512):
                    ps = ps_wo.tile([128, 512], F32)
                    for kc in range(H):
                        nc.tensor.matmul(
                            ps, attn_catT[:, kc, m * 128:(m + 1) * 128],
                            wo_sb[:, kc, n * 512:(n + 1) * 512],
                            start=(kc == 0), stop=(kc == H - 1))
                    nc.vector.tensor_add(x1_sb[:, n * 512:(n + 1) * 512], ps,
                                         xm_sb[:, n * 512:(n + 1) * 512])
                nc.sync.dma_start(out=x1_dram[m * 128:(m + 1) * 128, :],
                                  in_=x1_sb)
                scr2 = scr2_pool.tile([128, D], BF16)
                ssq2 = stat2_pool.tile([128, 1], F32, tag="ssq2")
                nc.scalar.activation(scr2, x1_sb, AF.Square, accum_out=ssq2)
                ms2 = stat2_pool.tile([128, 1], F32, tag="ms2")
                nc.scalar.activation(ms2, ssq2, AF.Identity,
                                     bias=eps_sb[:, 0:1], scale=1.0 / D)
                inv2 = stat2_pool.tile([128, 1], F32, tag="inv2")
                nc.vector.reciprocal(inv2, ms2)
                r2 = stat2_pool.tile([128, 1], F32, tag="r2")
                nc.scalar.activation(r2, inv2, AF.Sqrt)
                h2 = h2_pool.tile([128, D], BF16)
                nc.scalar.activation(h2, x1_sb, AF.Copy, scale=r2[:, 0:1])
                nc.sync.dma_start(out=h2_dram[m * 128:(m + 1) * 128, :], in_=h2)
            wo_psum_released = True
        pb.close()

        pf = contextlib.ExitStack()
        h2Tt_pool = pf.enter_context(tc.tile_pool(name="h2Tt", bufs=1))
        aT_pool = pf.enter_context(tc.tile_pool(name="aT", bufs=1))
        w1s_pool = pf.enter_context(tc.tile_pool(name="w1s", bufs=8))
        w2s_pool = pf.enter_context(tc.tile_pool(name="w2s", bufs=3))
        xr_pool = pf.enter_context(tc.tile_pool(name="xr", bufs=6))
        out_pool = pf.enter_context(tc.tile_pool(name="outsb", bufs=6))
        ps_f1 = pf.enter_context(tc.tile_pool(name="psf1", bufs=4, space="PSUM"))
        ps_f2 = pf.enter_context(tc.tile_pool(name="psf2", bufs=4, space="PSUM"))
        b2_pool = pf.enter_context(tc.tile_pool(name="b2p", bufs=1))
        if True:
            b2_sb = b2_pool.tile([128, D], F32)
            nc.sync.dma_start(out=b2_sb, in_=b2_ext[:])
            h2T_blks = [h2T_pool.tile([128, KD, TOKB], BF16, tag="h2T",
                                      name="h2T") for _ in range(NBLK)]
            for blk in range(NBLK):
                for d in range(KD):
                    nc.scalar.dma_start(
                        out=h2T_blks[blk][:, d, :],
                        in_=h2_dram[blk * TOKB:(blk + 1) * TOKB,
                                    d * 128:(d + 1) * 128], transpose=True)
            for blk in range(NBLK):
                aT = aT_pool.tile([128, FT, TOKB], BF16)
                for fs in range(DFF // 512):
                    ps4 = [ps_f1.tile([128, TOKB], F32, tag="psf1", name="psf1")
                           for _ in range(4)]
                    w1cs = []
                    for kq in range(KD // 4):
                        w1c = w1s_pool.tile([128, 4, 512], BF16, tag="w1c",
                                            name="w1c")
                        nc.scalar.dma_start(
                            out=w1c,
                            in_=w1_ext[kq * 512:(kq + 1) * 512,
                                       fs * 512:(fs + 1) * 512]
                            .rearrange("(k p) c -> p k c", p=128))
                        w1cs.append(w1c)
                    # one PSUM bank at a time: 16 consecutive MMs per group
                    for fsub in range(4):
                        for kq in range(KD // 4):
                            for kk in range(4):
                                kd = kq * 4 + kk
                                nc.tensor.matmul(
                                    ps4[fsub],
                                    w1cs[kq][:, kk, fsub * 128:(fsub + 1) * 128],
                                    h2T_blks[blk][:, kd, :],
                                    start=(kd == 0), stop=(kd == KD - 1))
                    for fsub in range(4):
                        f = fs * 4 + fsub
                        nc.scalar.activation(aT[:, f, :], ps4[fsub], AF.Relu,
                                             bias=b1_sb[:, f:f + 1])
                nmt = TOKB // 128
                for n in range(D // 512):
                    sl = slice(n * 512, (n + 1) * 512)
                    psm = [ps_f2.tile([128, 512], F32, tag="psf2", name="psf2")
                           for _ in range(nmt)]
                    for fq in range(FT // 4):
                        w2c = w2s_pool.tile([128, 4, 512], BF16)
                        nc.scalar.dma_start(
                            out=w2c,
                            in_=w2_ext[fq * 512:(fq + 1) * 512, sl]
                            .rearrange("(k p) c -> p k c", p=128))
                        for m in range(nmt):
                            for ff in range(4):
                                f = fq * 4 + ff
                                nc.tensor.matmul(
                                    psm[m], aT[:, f, m * 128:(m + 1) * 128],
                                    w2c[:, ff, :],
                                    start=(f == 0), stop=(f == FT - 1))
                    for m in range(nmt):
                        row0 = blk * TOKB + m * 128
                        xr = xr_pool.tile([128, 512], F32)
                        nc.sync.dma_start(out=xr,
                                          in_=x1_dram[row0:row0 + 128, sl])
                        osb = out_pool.tile([128, 512], F32)
                        nc.vector.tensor_add(osb, psm[m], xr)
                        nc.vector.tensor_add(osb, osb, b2_sb[:, sl])
                        nc.sync.dma_start(out=out_ext[row0:row0 + 128, sl],
                                          in_=osb)
        pf.close()
        ctx.close()
    nc.compile()
    return nc


def prep_inputs(cfg: Cfg, x, wq, wk, wv, wo, bo, w1, b1, w2, b2, g1, g2):
    """Host-side prep: fold gains, cast, relayout. Returns in_maps list."""
    B, S, D, HD, DFF = cfg.B, cfg.S, cfg.D, cfg.HD, cfg.DFF
    T, MYT, HPC, KD, FT = cfg.T, cfg.MYT, cfg.HPC, cfg.KD, cfg.FT
    bf = ml_dtypes.bfloat16

    xf = np.ascontiguousarray(x.reshape(T, D).astype(np.float32))
    wq_f = (wq * g1[None, :, None]).astype(bf)   # [H, D, HD]
    wk_f = (wk * g1[None, :, None]).astype(bf)
    wv_f = (wv * g1[None, :, None]).astype(bf)
    # [H, D, HD] -> per-core [128, HPC, KD, HD]
    def wslice(w, c):
        ws = w[c * HPC:(c + 1) * HPC]                    # [HPC, D, HD]
        ws = ws.reshape(HPC, KD, 128, HD).transpose(2, 0, 1, 3)
        return np.ascontiguousarray(ws)
    wo_l = np.ascontiguousarray(
        wo.astype(bf).reshape(KD, 128, D).transpose(1, 0, 2))  # [128, KD, D]
    w1_f = np.ascontiguousarray((w1 * g2[:, None]).astype(bf))
    w2_f = np.ascontiguousarray(w2.astype(bf))
    b1_c = np.ascontiguousarray(
        b1.astype(np.float32).reshape(FT, 128).T)        # [128, FT]
    b2_bc = np.broadcast_to(b2.astype(np.float32), (128, D)).copy()

    iidx = np.arange(128)[:, None]
    cidx = np.arange(896)[None, :]
    mask_big = (iidx <= cidx - 384).astype(bf)           # [128, 896]

    MYB = MYT // B
    in_maps = []
    for c in range(NC_CORES):
        rows = np.concatenate([np.arange(b * S + c * MYB, b * S + (c + 1) * MYB)
                               for b in range(B)])
        xm = (xf[rows] + bo.astype(np.float32)[None, :]).astype(np.float32)
        in_maps.append({
            "x": xf, "xmine": np.ascontiguousarray(xm),
            "wq": wslice(wq_f, c), "wk": wslice(wk_f, c), "wv": wslice(wv_f, c),
            "wo": wo_l, "w1": w1_f, "w2": w2_f,
            "b1c": b1_c, "b2bc": b2_bc, "maskbig": mask_big,
        })
    return in_maps


_NC_CACHE = {}


def _get_nc(cfg: Cfg):
    key = (cfg.B, cfg.S, cfg.D, cfg.H, cfg.HD, cfg.DFF)
    if key not in _NC_CACHE:
        _NC_CACHE[key] = build_nc(cfg)
    return _NC_CACHE[key]


def run(cfg: Cfg, inputs: dict, **spmd_kwargs):
    nc = _get_nc(cfg)
    in_maps = prep_inputs(cfg, **inputs)
    res = run_bass_kernel_spmd(nc, in_maps, list(range(NC_CORES)), **spmd_kwargs)
    MYB = cfg.MYT // cfg.B
    full = np.empty((cfg.T, cfg.D), np.float32)
    for c in range(NC_CORES):
        o = res.results[c]["out"]
        for b in range(cfg.B):
            full[b * cfg.S + c * MYB:b * cfg.S + (c + 1) * MYB] = \
                o[b * MYB:(b + 1) * MYB]
    return full.reshape(cfg.B, cfg.S, cfg.D), res


def kernel(**inputs) -> np.ndarray:
    cfg = Cfg()
    full, _ = run(cfg, {k: np.asarray(v) for k, v in inputs.items()})
    return full.astype(np.float32)



# revision 9
# speedup vs baseline: 1.1349x; 1.1349x over previous
"""Trainium2 Bass kernel for a dense transformer block (B=4, S=2048, D=2048,
H=16, HD=128, FFN 4x) on 8 NeuronCores.

Sharding: head-parallel attention (2 heads/core), per-(batch,head) AllToAll
(scatter tokens / gather heads), then token-parallel wo + FFN
(1024 tokens/core). Matmuls in bf16, fp32 PSUM accumulation.
RMSNorm gains g1/g2 folded into wq/wk/wv/w1 on the host.

Engine assignment (all queues are in-order, so placement matters):
  sync   : x?? no - h1 writes, h1T transposes, phase-B loads/stores
  gpsimd : x loads (fp32->bf16 cast), a2a copies+triggers, catT gathers
  scalar : softmax exp (paired [128,1024] tiles), FFN weight streams, relu
  vector : RMSNorm (square-accumulate + Newton rsqrt), psum copies, masks,
           softmax denominator reciprocal (approx), residual adds
  PE     : all matmuls; batches software-pipelined so PE never waits on norm
"""

import numpy as np
import ml_dtypes

import concourse.bass as bass
import concourse.tile as tile
import concourse.mybir as mybir
from concourse import bacc
from concourse.bass_utils import run_bass_kernel_spmd

F32 = mybir.dt.float32
BF16 = mybir.dt.bfloat16
AF = mybir.ActivationFunctionType
ALU = mybir.AluOpType

NC_CORES = 8

# bisect flags
X_CAST = False      # x loads via gpsimd with fp32->bf16 cast (else sync fp32)
BIG_XPOSE = False   # [2048,128] h1T transposes (else [512,128] blockwise)
A2A_SPLIT = False   # per-(batch,head) a2a (else per-batch, baseline style)
NEW_DVE_OPS = False # tensor_tensor_reduce/tensor_scalar norm chains on DVE
RECIP_FAST = False  # reciprocal_approx_fast for softmax denominators


class Cfg:
    def __init__(self, B=4, S=2048, D=2048, H=16, HD=128, DFF=8192):
        self.B, self.S, self.D, self.H, self.HD, self.DFF = B, S, D, H, HD, DFF
        self.T = B * S                      # total tokens
        self.MYT = self.T // NC_CORES       # tokens per core
        self.HPC = H // NC_CORES            # heads per core
        self.KD = D // 128                  # D partition-tiles
        self.FT = DFF // 128                # ffn partition-tiles
        assert self.HPC * NC_CORES == H and HD == 128 and D == H * HD
        assert self.MYT % 128 == 0 and S % 512 == 0 and DFF % 512 == 0
        self.TOKB = min(512, self.MYT)      # ffn token-block
        self.NBLK = self.MYT // self.TOKB


def build_nc(cfg: Cfg):
    B, S, D, HD, DFF = cfg.B, cfg.S, cfg.D, cfg.HD, cfg.DFF
    T, MYT, HPC, KD, FT = cfg.T, cfg.MYT, cfg.HPC, cfg.KD, cfg.FT
    TOKB, NBLK = cfg.TOKB, cfg.NBLK
    H = cfg.H
    SCALE = float(HD) ** -0.5
    import contextlib

    nc = bacc.Bacc("TRN2", target_bir_lowering=False, debug=False,
                   num_devices=NC_CORES)

    x_ext = nc.dram_tensor("x", [T, D], F32, kind="ExternalInput")
    xm_ext = nc.dram_tensor("xmine", [MYT, D], F32, kind="ExternalInput")
    wq_ext = nc.dram_tensor("wq", [128, HPC, KD, HD], BF16, kind="ExternalInput")
    wk_ext = nc.dram_tensor("wk", [128, HPC, KD, HD], BF16, kind="ExternalInput")
    wv_ext = nc.dram_tensor("wv", [128, HPC, KD, HD], BF16, kind="ExternalInput")
    wo_ext = nc.dram_tensor("wo", [128, KD, D], BF16, kind="ExternalInput")
    w1_ext = nc.dram_tensor("w1", [D, DFF], BF16, kind="ExternalInput")
    w2_ext = nc.dram_tensor("w2", [DFF, D], BF16, kind="ExternalInput")
    b1_ext = nc.dram_tensor("b1c", [128, FT], F32, kind="ExternalInput")
    b2_ext = nc.dram_tensor("b2bc", [128, D], F32, kind="ExternalInput")
    mask_ext = nc.dram_tensor("maskbig", [128, 896], BF16, kind="ExternalInput")
    out_ext = nc.dram_tensor("out", [MYT, D], F32, kind="ExternalOutput")

    MYB = MYT // B          # my tokens per batch
    if A2A_SPLIT:
        a2a_in = [[nc.dram_tensor(f"a2a_in{b}_{h}", [NC_CORES, 128, MYB], BF16)
                   for h in range(HPC)] for b in range(B)]
        a2a_out = [[nc.dram_tensor(f"a2a_out{b}_{h}", [NC_CORES, 128, MYB],
                                   BF16) for h in range(HPC)] for b in range(B)]
    else:
        a2a_in = [nc.dram_tensor(f"a2a_in{b}", [NC_CORES, HPC, 128, MYB], BF16)
                  for b in range(B)]
        a2a_out = [nc.dram_tensor(f"a2a_out{b}", [NC_CORES, HPC, 128, MYB],
                                  BF16) for b in range(B)]
    h1_dram = nc.dram_tensor("h1_spill", [T, D], BF16)
    h2_dram = nc.dram_tensor("h2_spill", [MYT, D], BF16)
    x1_dram = nc.dram_tensor("x1_spill", [MYT, D], F32)

    with tile.TileContext(nc) as tc:
        ctx = contextlib.ExitStack()
        const = ctx.enter_context(tc.tile_pool(name="const", bufs=1))
        catT_pool = ctx.enter_context(tc.tile_pool(name="catT", bufs=1))

        mask_sb = const.tile([128, 896], BF16)
        nc.sync.dma_start(out=mask_sb, in_=mask_ext[:])
        ones_sb = const.tile([128, 128], BF16)
        nc.vector.memset(ones_sb, 1.0)
        b1_sb = const.tile([128, FT], F32)
        nc.sync.dma_start(out=b1_sb, in_=b1_ext[:])
        eps_sb = const.tile([128, 1], F32)
        nc.vector.memset(eps_sb, 1e-8)

        attn_catT = catT_pool.tile([128, H, MYT], BF16)

        # ---------------- Phase A: norm1 + QKV + attention ----------------
        pa = contextlib.ExitStack()
        qkvw_pool = pa.enter_context(tc.tile_pool(name="qkvw", bufs=1))
        aout_pool = pa.enter_context(tc.tile_pool(name="aout", bufs=1))
        xa_pool = pa.enter_context(tc.tile_pool(name="xa", bufs=3))
        scr_pool = pa.enter_context(tc.tile_pool(name="scr", bufs=1))
        stat_pool = pa.enter_context(tc.tile_pool(name="stat", bufs=2))
        h1_pool = pa.enter_context(tc.tile_pool(name="h1", bufs=2))
        h1T_pool = pa.enter_context(tc.tile_pool(name="h1T", bufs=1))
        qk_pool = pa.enter_context(tc.tile_pool(name="qkT", bufs=1))
        v_pool = pa.enter_context(tc.tile_pool(name="vtok", bufs=1))
        p_pool = pa.enter_context(tc.tile_pool(name="pT", bufs=3))
        pm_pool = pa.enter_context(tc.tile_pool(name="pTm", bufs=2))
        dd_pool = pa.enter_context(tc.tile_pool(name="dd", bufs=2))
        ps1024 = pa.enter_context(tc.tile_pool(name="ps1024", bufs=2, space="PSUM"))
        ps_v = pa.enter_context(tc.tile_pool(name="psv", bufs=1, space="PSUM"))
        ps_av = pa.enter_context(tc.tile_pool(name="psav", bufs=2, space="PSUM"))
        ps_den = pa.enter_context(tc.tile_pool(name="psden", bufs=1, space="PSUM"))

        wq_sb = qkvw_pool.tile([128, HPC, KD, HD], BF16)
        wk_sb = qkvw_pool.tile([128, HPC, KD, HD], BF16)
        wv_sb = qkvw_pool.tile([128, HPC, KD, HD], BF16)
        nc.sync.dma_start(out=wq_sb, in_=wq_ext[:])
        nc.sync.dma_start(out=wk_sb, in_=wk_ext[:])
        nc.sync.dma_start(out=wv_sb, in_=wv_ext[:])

        # per-batch tiles, rotated via pools
        h1T_t = {}
        qkT_t = {}
        v_t = {}
        aout_t = {}

        def emit_norm(b, blockwise):
            """x loads (gpsimd, cast), RMSNorm on DVE, h1 writes + transposes."""
            h1T = h1T_pool.tile([128, KD, S], BF16, tag="h1T", name="h1T")
            h1T_t[b] = h1T
            if not NEW_DVE_OPS:
                for nb in range(4):
                    for t4 in range(4):
                        row0 = b * S + nb * 512 + t4 * 128
                        x_sb = xa_pool.tile([128, D], F32, tag="xf32")
                        nc.sync.dma_start(out=x_sb,
                                          in_=x_ext[row0:row0 + 128, :])
                        scr_t = scr_pool.tile([128, D], BF16)
                        ssq = stat_pool.tile([128, 1], F32, tag="ssq")
                        nc.scalar.activation(scr_t, x_sb, AF.Square,
                                             accum_out=ssq)
                        ms = stat_pool.tile([128, 1], F32, tag="ms")
                        nc.scalar.activation(ms, ssq, AF.Identity,
                                             bias=eps_sb[:, 0:1],
                                             scale=1.0 / D)
                        inv = stat_pool.tile([128, 1], F32, tag="inv")
                        nc.vector.reciprocal(inv, ms)
                        rr = stat_pool.tile([128, 1], F32, tag="rr")
                        nc.scalar.activation(rr, inv, AF.Sqrt)
                        h1 = h1_pool.tile([128, D], BF16)
                        nc.scalar.activation(h1, x_sb, AF.Copy,
                                             scale=rr[:, 0:1])
                        nc.sync.dma_start(out=h1_dram[row0:row0 + 128, :],
                                          in_=h1)
                    if blockwise:
                        tok0 = b * S + nb * 512
                        for d in range(KD):
                            nc.sync.dma_start(
                                out=h1T[:, d, nb * 512:(nb + 1) * 512],
                                in_=h1_dram[tok0:tok0 + 512,
                                            d * 128:(d + 1) * 128],
                                transpose=True)
                if not blockwise:
                    for d in range(KD):
                        nc.sync.dma_start(
                            out=h1T[:, d, :],
                            in_=h1_dram[b * S:(b + 1) * S,
                                        d * 128:(d + 1) * 128], transpose=True)
                return
            G = 2   # row-tiles per stats group (keeps x-tile pool small)
            for nb in range(4):
                for g in range(4 // G):
                    xs = []
                    ssq = stat_pool.tile([128, G], F32, tag="ssq")
                    for t4 in range(g * G, (g + 1) * G):
                        row0 = b * S + nb * 512 + t4 * 128
                        if X_CAST:
                            x_sb = xa_pool.tile([128, D], BF16)
                            nc.gpsimd.dma_start(out=x_sb,
                                                in_=x_ext[row0:row0 + 128, :])
                        else:
                            x_sb = xa_pool.tile([128, D], F32, tag="xf32")
                            nc.sync.dma_start(out=x_sb,
                                              in_=x_ext[row0:row0 + 128, :])
                        scr_t = scr_pool.tile([128, D], BF16)
                        nc.vector.tensor_tensor_reduce(
                            out=scr_t, in0=x_sb, in1=x_sb, scale=1.0,
                            scalar=0.0, op0=ALU.mult, op1=ALU.add,
                            accum_out=ssq[:, t4 - g * G:t4 - g * G + 1])
                        xs.append(x_sb)
                    # r = rsqrt(ssq/D + eps): seed + 2 Newton iterations (DVE)
                    ms = stat_pool.tile([128, G], F32, tag="ms")
                    nc.vector.tensor_scalar(out=ms, in0=ssq, scalar1=1.0 / D,
                                            scalar2=1e-8, op0=ALU.mult,
                                            op1=ALU.add)
                    r = stat_pool.tile([128, G], F32, tag="r0")
                    nc.vector.tensor_scalar(out=r, in0=ms, scalar1=-0.5,
                                            scalar2=1.5, op0=ALU.mult,
                                            op1=ALU.add)
                    for it in range(2):
                        a_ = stat_pool.tile([128, G], F32, tag=f"nra{it}")
                        nc.vector.tensor_tensor(out=a_, in0=r, in1=r,
                                                op=ALU.mult)
                        bb = stat_pool.tile([128, G], F32, tag=f"nrb{it}")
                        nc.vector.tensor_tensor(out=bb, in0=ms, in1=a_,
                                                op=ALU.mult)
                        cc = stat_pool.tile([128, G], F32, tag=f"nrc{it}")
                        nc.vector.tensor_scalar(out=cc, in0=bb, scalar1=-0.5,
                                                scalar2=1.5, op0=ALU.mult,
                                                op1=ALU.add)
                        r2 = stat_pool.tile([128, G], F32, tag=f"nrr{it}")
                        nc.vector.tensor_tensor(out=r2, in0=r, in1=cc,
                                                op=ALU.mult)
                        r = r2
                    for t4 in range(g * G, (g + 1) * G):
                        row0 = b * S + nb * 512 + t4 * 128
                        h1 = h1_pool.tile([128, D], BF16)
                        nc.vector.tensor_scalar_mul(h1, xs[t4 - g * G],
                                                    r[:, t4 - g * G:
                                                      t4 - g * G + 1])
                        nc.sync.dma_start(out=h1_dram[row0:row0 + 128, :],
                                          in_=h1)
                if blockwise:
                    tok0 = b * S + nb * 512
                    for d in range(KD):
                        nc.sync.dma_start(
                            out=h1T[:, d, nb * 512:(nb + 1) * 512],
                            in_=h1_dram[tok0:tok0 + 512,
                                        d * 128:(d + 1) * 128], transpose=True)
            if not blockwise:
                for d in range(KD):
                    nc.sync.dma_start(
                        out=h1T[:, d, :],
                        in_=h1_dram[b * S:(b + 1) * S,
                                    d * 128:(d + 1) * 128], transpose=True)

        def emit_qkv(b):
            h1T = h1T_t[b]
            qkT = [qk_pool.tile([128, 2, S], BF16, tag=f"qkT{h}",
                                name="qkT") for h in range(HPC)]
            qkT_t[b] = qkT
            v_sb = [v_pool.tile([128, S // 128, 128], BF16, tag=f"v{h}",
                                name="v") for h in range(HPC)]
            v_t[b] = v_sb
            for nb in range(4):
                sl = slice(nb * 512, (nb + 1) * 512)
                for h in range(HPC):
                    ps = ps1024.tile([128, 1024], F32, tag="ps1024")
                    for kd in range(KD):
                        nc.tensor.matmul(ps[:, 0:512], wq_sb[:, h, kd, :],
                                         h1T[:, kd, sl],
                                         start=(kd == 0), stop=(kd == KD - 1))
                    for kd in range(KD):
                        nc.tensor.matmul(ps[:, 512:1024], wk_sb[:, h, kd, :],
                                         h1T[:, kd, sl],
                                         start=(kd == 0), stop=(kd == KD - 1))
                    nc.vector.tensor_copy(qkT[h][:, :, sl], ps)
                for sk4 in range(4):
                    psv = ps_v.tile([128, HPC * HD], F32)
                    col = slice(nb * 512 + sk4 * 128, nb * 512 + sk4 * 128 + 128)
                    for kd in range(KD):
                        nc.tensor.matmul(
                            psv, h1T[:, kd, col], wv_sb[:, :, kd, :],
                            start=(kd == 0), stop=(kd == KD - 1))
                    for h in range(HPC):
                        nc.vector.tensor_copy(
                            v_sb[h][:, nb * 4 + sk4, :],
                            psv[:, h * HD:(h + 1) * HD])

        def emit_attention(b):
            qkT, v_sb = qkT_t[b], v_t[b]
            attn_outT = [aout_pool.tile([128, S], BF16, tag=f"aout{h}",
                                        name="attn_outT") for h in range(HPC)]
            aout_t[b] = attn_outT
            for h in range(HPC):
                qT = qkT[h][:, 0, :]
                kT = qkT[h][:, 1, :]
                for q4 in range(S // 512):
                    sq = q4 * 512
                    n_alive = q4 * 4 + 4
                    av_ps = ps_av.tile([128, 512], F32)
                    den_ps = ps_den.tile([128, 512], F32)
                    prev = None

                    def flush(pair):
                        uses, pk0 = pair
                        for half in (0, 1):
                            sk = 2 * pk0 + half
                            nc.tensor.matmul(av_ps, v_sb[h][:, sk, :],
                                             uses[half],
                                             start=(sk == 0),
                                             stop=(sk == n_alive - 1))
                            nc.tensor.matmul(den_ps, ones_sb, uses[half],
                                             start=(sk == 0),
                                             stop=(sk == n_alive - 1))

                    for pk in range(n_alive // 2):
                        sc = ps1024.tile([128, 1024], F32, tag="ps1024")
                        for half in (0, 1):
                            sk = 2 * pk + half
                            nc.tensor.matmul(
                                sc[:, half * 512:(half + 1) * 512],
                                kT[:, sk * 128:(sk + 1) * 128],
                                qT[:, sq:sq + 512], start=True, stop=True)
                        if prev is not None:
                            flush(prev)
                        pT = p_pool.tile([128, 1024], BF16)
                        nc.scalar.activation(pT, sc, AF.Exp, scale=SCALE)
                        uses = []
                        for half in (0, 1):
                            sk = 2 * pk + half
                            u = pT[:, half * 512:(half + 1) * 512]
                            delta = sq - sk * 128
                            if delta < 128:  # partial (diag-band) tile
                                o = delta + 384
                                pTm = pm_pool.tile([128, 512], BF16)
                                nc.vector.tensor_mul(pTm, u,
                                                     mask_sb[:, o:o + 512])
                                u = pTm
                            uses.append(u)
                        prev = (uses, pk)
                    flush(prev)
                    dinv = dd_pool.tile([128, 512], F32, tag="dinv")
                    if RECIP_FAST:
                        nc.vector.reciprocal_approx_fast(out=dinv, in_=den_ps)
                    else:
                        nc.vector.reciprocal(dinv, den_ps)
                    nc.vector.tensor_mul(attn_outT[h][:, sq:sq + 512],
                                         av_ps, dinv)
                if A2A_SPLIT:
                    # ship this head now: overlaps the rest of the batch
                    for j in range(NC_CORES):
                        nc.gpsimd.dma_start(
                            out=a2a_in[b][h][j],
                            in_=attn_outT[h][:, j * MYB:(j + 1) * MYB])
                    nc.gpsimd.collective_compute(
                        "AllToAll", ALU.bypass,
                        replica_groups=[list(range(NC_CORES))],
                        ins=[a2a_in[b][h][:].opt()],
                        outs=[a2a_out[b][h][:].opt()])
            if not A2A_SPLIT:
                for h in range(HPC):
                    for j in range(NC_CORES):
                        nc.sync.dma_start(
                            out=a2a_in[b][j, h],
                            in_=attn_outT[h][:, j * MYB:(j + 1) * MYB])
                nc.gpsimd.collective_compute(
                    "AllToAll", ALU.bypass,
                    replica_groups=[list(range(NC_CORES))],
                    ins=[a2a_in[b][:].opt()], outs=[a2a_out[b][:].opt()])

        def emit_gather(b):
            if A2A_SPLIT:
                for c in range(NC_CORES):
                    for h in range(HPC):
                        kc = c * HPC + h
                        nc.gpsimd.dma_start(
                            out=attn_catT[:, kc, b * MYB:(b + 1) * MYB],
                            in_=a2a_out[b][h][c])
            else:
                flat = a2a_out[b].ap().rearrange("c h p t -> (c h) p t")
                for kc in range(H):
                    nc.gpsimd.dma_start(
                        out=attn_catT[:, kc, b * MYB:(b + 1) * MYB],
                        in_=flat[kc])

        # software-pipelined emission
        emit_norm(0, blockwise=True)
        emit_qkv(0)
        emit_norm(1, blockwise=not BIG_XPOSE)
        emit_attention(0)
        emit_qkv(1)
        emit_norm(2, blockwise=not BIG_XPOSE)
        emit_attention(1)
        emit_gather(0)
        emit_qkv(2)
        emit_norm(3, blockwise=not BIG_XPOSE)
        emit_attention(2)
        emit_gather(1)
        emit_qkv(3)

        emit_attention(3)
        emit_gather(2)
        emit_gather(3)
        pa.close()

        # FFN-spanning pools (created now that phase-A space is free)
        h2T_pool = ctx.enter_context(tc.tile_pool(name="h2T", bufs=NBLK))
        w1pre_pool = ctx.enter_context(tc.tile_pool(name="w1pre", bufs=4))
        w1_pre = {}
        for fs in range(1):
            w1cs = []
            for kq in range(KD // 4):
                w1c = w1pre_pool.tile([128, 4, 512], BF16, tag="w1c",
                                      name="w1c")
                nc.scalar.dma_start(
                    out=w1c,
                    in_=w1_ext[kq * 512:(kq + 1) * 512,
                               fs * 512:(fs + 1) * 512]
                    .rearrange("(k p) c -> p k c", p=128))
                w1cs.append(w1c)
            w1_pre[(0, fs)] = w1cs

        # ---------------- Phase B: wo + residual + norm2 (fused per m) -----
        b2_pool = ctx.enter_context(tc.tile_pool(name="b2p", bufs=1))
        pb = contextlib.ExitStack()
        wo_pool = pb.enter_context(tc.tile_pool(name="wo", bufs=1))
        xb_pool = pb.enter_context(tc.tile_pool(name="xb", bufs=2))
        x1_pool = pb.enter_context(tc.tile_pool(name="x1", bufs=2))
        stat2_pool = pb.enter_context(tc.tile_pool(name="stat2", bufs=2))
        scr2_pool = pb.enter_context(tc.tile_pool(name="scr2", bufs=1))
        h2_pool = pb.enter_context(tc.tile_pool(name="h2", bufs=2))
        ps_wo = pb.enter_context(tc.tile_pool(name="pswo", bufs=2, space="PSUM"))

        wo_sb = wo_pool.tile([128, KD, D], BF16)
        for kc in range(KD):
            nc.sync.dma_start(out=wo_sb[:, kc, :], in_=wo_ext[:, kc, :])
        b2_sb = b2_pool.tile([128, D], F32)
        nc.sync.dma_start(out=b2_sb, in_=b2_ext[:])

        h2T_blks = [h2T_pool.tile([128, KD, TOKB], BF16, tag="h2T",
                                  name="h2T") for _ in range(NBLK)]

        for m in range(MYT // 128):
            xm_sb = xb_pool.tile([128, D], F32)
            nc.sync.dma_start(out=xm_sb, in_=xm_ext[m * 128:(m + 1) * 128, :])
            x1_sb = x1_pool.tile([128, D], F32)
            for n in range(D // 512):
                ps = ps_wo.tile([128, 512], F32)
                for kc in range(H):
                    nc.tensor.matmul(
                        ps, attn_catT[:, kc, m * 128:(m + 1) * 128],
                        wo_sb[:, kc, n * 512:(n + 1) * 512],
                        start=(kc == 0), stop=(kc == H - 1))
                nc.vector.tensor_add(x1_sb[:, n * 512:(n + 1) * 512], ps,
                                     xm_sb[:, n * 512:(n + 1) * 512])
            nc.sync.dma_start(out=x1_dram[m * 128:(m + 1) * 128, :],
                              in_=x1_sb)
            # norm2
            scr2 = scr2_pool.tile([128, D], BF16)
            ssq2 = stat2_pool.tile([128, 1], F32, tag="ssq2")
            h2 = h2_pool.tile([128, D], BF16)
            if NEW_DVE_OPS:
                nc.vector.tensor_tensor_reduce(
                    out=scr2, in0=x1_sb, in1=x1_sb, scale=1.0, scalar=0.0,
                    op0=ALU.mult, op1=ALU.add, accum_out=ssq2)
                ms2 = stat2_pool.tile([128, 1], F32, tag="ms2")
                nc.vector.tensor_scalar(out=ms2, in0=ssq2, scalar1=1.0 / D,
                                        scalar2=1e-8, op0=ALU.mult,
                                        op1=ALU.add)
                r2 = stat2_pool.tile([128, 1], F32, tag="r2s")
                nc.vector.tensor_scalar(out=r2, in0=ms2, scalar1=-0.5,
                                        scalar2=1.5, op0=ALU.mult,
                                        op1=ALU.add)
                for it in range(2):
                    a_ = stat2_pool.tile([128, 1], F32, tag=f"n2a{it}")
                    nc.vector.tensor_tensor(out=a_, in0=r2, in1=r2,
                                            op=ALU.mult)
                    bb = stat2_pool.tile([128, 1], F32, tag=f"n2b{it}")
                    nc.vector.tensor_tensor(out=bb, in0=ms2, in1=a_,
                                            op=ALU.mult)
                    cc = stat2_pool.tile([128, 1], F32, tag=f"n2c{it}")
                    nc.vector.tensor_scalar(out=cc, in0=bb, scalar1=-0.5,
                                            scalar2=1.5, op0=ALU.mult,
                                            op1=ALU.add)
                    r2n = stat2_pool.tile([128, 1], F32, tag=f"n2r{it}")
                    nc.vector.tensor_tensor(out=r2n, in0=r2, in1=cc,
                                            op=ALU.mult)
                    r2 = r2n
                nc.vector.tensor_scalar_mul(h2, x1_sb, r2[:, 0:1])
            else:
                nc.scalar.activation(scr2, x1_sb, AF.Square, accum_out=ssq2)
                ms2 = stat2_pool.tile([128, 1], F32, tag="ms2")
                nc.scalar.activation(ms2, ssq2, AF.Identity,
                                     bias=eps_sb[:, 0:1], scale=1.0 / D)
                inv2 = stat2_pool.tile([128, 1], F32, tag="inv2")
                nc.vector.reciprocal(inv2, ms2)
                rr2 = stat2_pool.tile([128, 1], F32, tag="rr2")
                nc.scalar.activation(rr2, inv2, AF.Sqrt)
                nc.scalar.activation(h2, x1_sb, AF.Copy, scale=rr2[:, 0:1])
            nc.sync.dma_start(out=h2_dram[m * 128:(m + 1) * 128, :], in_=h2)
            # kick off h2T transposes as soon as a block's 4 m-tiles exist
            blk = m // (TOKB // 128)
            if (m + 1) % (TOKB // 128) == 0:
                for d in range(KD):
                    nc.scalar.dma_start(
                        out=h2T_blks[blk][:, d, :],
                        in_=h2_dram[blk * TOKB:(blk + 1) * TOKB,
                                    d * 128:(d + 1) * 128], transpose=True)
        pb.close()

        # ---------------- FFN ----------------
        pf = contextlib.ExitStack()
        aT_pool = pf.enter_context(tc.tile_pool(name="aT", bufs=1))
        w1s_pool = pf.enter_context(tc.tile_pool(name="w1s", bufs=4))
        w2s_pool = pf.enter_context(tc.tile_pool(name="w2s", bufs=3))
        xr_pool = pf.enter_context(tc.tile_pool(name="xr", bufs=4))
        out_pool = pf.enter_context(tc.tile_pool(name="outsb", bufs=4))
        ps_f1 = pf.enter_context(tc.tile_pool(name="psf1", bufs=4, space="PSUM"))
        ps_f2 = pf.enter_context(tc.tile_pool(name="psf2", bufs=4, space="PSUM"))

        for blk in range(NBLK):
            aT = aT_pool.tile([128, FT, TOKB], BF16)
            for fs in range(DFF // 512):
                ps4 = [ps_f1.tile([128, TOKB], F32, tag="psf1", name="psf1")
                       for _ in range(4)]
                if (blk, fs) in w1_pre:
                    w1cs = w1_pre[(blk, fs)]
                else:
                    w1cs = []
                    for kq in range(KD // 4):
                        w1c = w1s_pool.tile([128, 4, 512], BF16, tag="w1c",
                                            name="w1c")
                        nc.scalar.dma_start(
                            out=w1c,
                            in_=w1_ext[kq * 512:(kq + 1) * 512,
                                       fs * 512:(fs + 1) * 512]
                            .rearrange("(k p) c -> p k c", p=128))
                        w1cs.append(w1c)
                # one PSUM bank at a time: 16 consecutive MMs per group
                for fsub in range(4):
                    for kq in range(KD // 4):
                        for kk in range(4):
                            kd = kq * 4 + kk
                            nc.tensor.matmul(
                                ps4[fsub],
                                w1cs[kq][:, kk, fsub * 128:(fsub + 1) * 128],
                                h2T_blks[blk][:, kd, :],
                                start=(kd == 0), stop=(kd == KD - 1))
                for fsub in range(4):
                    f = fs * 4 + fsub
                    nc.scalar.activation(aT[:, f, :], ps4[fsub], AF.Relu,
                                         bias=b1_sb[:, f:f + 1])
            nmt = TOKB // 128
            for n in range(D // 512):
                sl = slice(n * 512, (n + 1) * 512)
                psm = [ps_f2.tile([128, 512], F32, tag="psf2", name="psf2")
                       for _ in range(nmt)]
                for fq in range(FT // 4):
                    w2c = w2s_pool.tile([128, 4, 512], BF16)
                    nc.scalar.dma_start(
                        out=w2c,
                        in_=w2_ext[fq * 512:(fq + 1) * 512, sl]
                        .rearrange("(k p) c -> p k c", p=128))
                    for m in range(nmt):
                        for ff in range(4):
                            f = fq * 4 + ff
                            nc.tensor.matmul(
                                psm[m], aT[:, f, m * 128:(m + 1) * 128],
                                w2c[:, ff, :],
                                start=(f == 0), stop=(f == FT - 1))
                for m in range(nmt):
                    row0 = blk * TOKB + m * 128
                    xr = xr_pool.tile([128, 512], F32)
                    nc.sync.dma_start(out=xr,
                                      in_=x1_dram[row0:row0 + 128, sl])
                    osb = out_pool.tile([128, 512], F32)
                    nc.vector.tensor_add(osb, psm[m], xr)
                    nc.vector.tensor_add(osb, osb, b2_sb[:, sl])
                    nc.sync.dma_start(out=out_ext[row0:row0 + 128, sl],
                                      in_=osb)
        pf.close()
        ctx.close()
    nc.compile()
    return nc


def prep_inputs(cfg: Cfg, x, wq, wk, wv, wo, bo, w1, b1, w2, b2, g1, g2):
    """Host-side prep: fold gains, cast, relayout. Returns in_maps list."""
    B, S, D, HD, DFF = cfg.B, cfg.S, cfg.D, cfg.HD, cfg.DFF
    T, MYT, HPC, KD, FT = cfg.T, cfg.MYT, cfg.HPC, cfg.KD, cfg.FT
    bf = ml_dtypes.bfloat16

    xf = np.ascontiguousarray(x.reshape(T, D).astype(np.float32))
    wq_f = (wq * g1[None, :, None]).astype(bf)   # [H, D, HD]
    wk_f = (wk * g1[None, :, None]).astype(bf)
    wv_f = (wv * g1[None, :, None]).astype(bf)
    # [H, D, HD] -> per-core [128, HPC, KD, HD]
    def wslice(w, c):
        ws = w[c * HPC:(c + 1) * HPC]                    # [HPC, D, HD]
        ws = ws.reshape(HPC, KD, 128, HD).transpose(2, 0, 1, 3)
        return np.ascontiguousarray(ws)
    wo_l = np.ascontiguousarray(
        wo.astype(bf).reshape(KD, 128, D).transpose(1, 0, 2))  # [128, KD, D]
    w1_f = np.ascontiguousarray((w1 * g2[:, None]).astype(bf))
    w2_f = np.ascontiguousarray(w2.astype(bf))
    b1_c = np.ascontiguousarray(
        b1.astype(np.float32).reshape(FT, 128).T)        # [128, FT]
    b2_bc = np.broadcast_to(b2.astype(np.float32), (128, D)).copy()

    iidx = np.arange(128)[:, None]
    cidx = np.arange(896)[None, :]
    mask_big = (iidx <= cidx - 384).astype(bf)           # [128, 896]

    MYB = MYT // B
    in_maps = []
    for c in range(NC_CORES):
        rows = np.concatenate([np.arange(b * S + c * MYB, b * S + (c + 1) * MYB)
                               for b in range(B)])
        xm = (xf[rows] + bo.astype(np.float32)[None, :]).astype(np.float32)
        in_maps.append({
            "x": xf, "xmine": np.ascontiguousarray(xm),
            "wq": wslice(wq_f, c), "wk": wslice(wk_f, c), "wv": wslice(wv_f, c),
            "wo": wo_l, "w1": w1_f, "w2": w2_f,
            "b1c": b1_c, "b2bc": b2_bc, "maskbig": mask_big,
        })
    return in_maps


_NC_CACHE = {}


def _get_nc(cfg: Cfg):
    key = (cfg.B, cfg.S, cfg.D, cfg.H, cfg.HD, cfg.DFF)
    if key not in _NC_CACHE:
        _NC_CACHE[key] = build_nc(cfg)
    return _NC_CACHE[key]


def run(cfg: Cfg, inputs: dict, **spmd_kwargs):
    nc = _get_nc(cfg)
    in_maps = prep_inputs(cfg, **inputs)
    res = run_bass_kernel_spmd(nc, in_maps, list(range(NC_CORES)), **spmd_kwargs)
    MYB = cfg.MYT // cfg.B
    full = np.empty((cfg.T, cfg.D), np.float32)
    for c in range(NC_CORES):
        o = res.results[c]["out"]
        for b in range(cfg.B):
            full[b * cfg.S + c * MYB:b * cfg.S + (c + 1) * MYB] = \
                o[b * MYB:(b + 1) * MYB]
    return full.reshape(cfg.B, cfg.S, cfg.D), res


def kernel(**inputs) -> np.ndarray:
    cfg = Cfg()
    full, _ = run(cfg, {k: np.asarray(v) for k, v in inputs.items()})
    return full.astype(np.float32)
